# revision 1
# baseline (speedup 1.0000x reference)
"""Trainium2 Bass kernel for nn_HOPEProjection (LayerNorm -> MLP(2048->512,
GELU) -> Linear(512->96) -> tanh+1, split into 3 heads-tensors).

Contract: kernel(**inputs) takes the FULL inputs from setup_inputs() and
returns the FULL output (tuple of three [16384, 32] float32 arrays), running
the compute data-parallel across 8 NeuronCores.

Self-contained: hardcodes all shapes; does not read any sibling files.
"""

import sys

for _p in ("/opt/trn_rl_repo",):
    if _p not in sys.path:
        sys.path.append(_p)

import numpy as np
import ml_dtypes

import concourse.bacc as bacc
import concourse.mybir as mybir
import concourse.tile as tile
from concourse import bass_utils

# ---- problem constants (hardcoded per contract) ----
P = 128              # SBUF partitions
D = 2048             # d_model
H = 512              # hidden
C = 96               # 3 * n_heads
NH = 32              # n_heads
B = 16384            # batch
NCORES = 8
BS = B // NCORES     # rows per core = 2048
RCW = 512            # row-chunk width (matmul moving dim)
NRC = BS // RCW      # 4 row chunks per core
KC = D // P          # 16 contraction chunks
HT = H // P          # 4 hidden tiles
EPS = 1e-5

F32 = mybir.dt.float32
BF16 = mybir.dt.bfloat16
AF = mybir.ActivationFunctionType

_CACHE = {}


def _build_nc(with_b1=False):
    nc = bacc.Bacc("TRN2", target_bir_lowering=False, debug=False)

    xT = nc.dram_tensor("xT", [D, BS], F32, kind="ExternalInput").ap()
    w1 = nc.dram_tensor("w1", [D, H], BF16, kind="ExternalInput").ap()
    w2 = nc.dram_tensor("w2", [H, C], BF16, kind="ExternalInput").ap()
    cs2 = nc.dram_tensor("cs2", [2, H], BF16, kind="ExternalInput").ap()
    b1c = nc.dram_tensor("b1c", [P, HT], F32, kind="ExternalInput").ap()
    b2r = nc.dram_tensor("b2r", [1, C], BF16, kind="ExternalInput").ap()
    ones_col = nc.dram_tensor("ones_col", [P, 1], BF16, kind="ExternalInput").ap()
    ones_row_b = nc.dram_tensor("ones_row_b", [1, RCW], BF16, kind="ExternalInput").ap()
    ones_row_f = nc.dram_tensor("ones_row_f", [1, P], F32, kind="ExternalInput").ap()
    pT = nc.dram_tensor("pT", [C, BS], F32, kind="ExternalOutput").ap()

    with tile.TileContext(nc) as tc:
        _body(tc, xT, w1, w2, cs2, b1c, b2r, ones_col, ones_row_b, ones_row_f, pT, with_b1)
    nc.compile()
    return nc


def _body(tc, xT, w1, w2, cs2, b1c, b2r, ones_col, ones_row_b, ones_row_f, pT, with_b1):
    nc = tc.nc
    import contextlib

    ctx = contextlib.ExitStack()
    with ctx:
        const = ctx.enter_context(tc.tile_pool(name="const", bufs=1))
        xbp = ctx.enter_context(tc.tile_pool(name="xb", bufs=NRC))
        x2p = ctx.enter_context(tc.tile_pool(name="x2", bufs=1))
        trp = ctx.enter_context(tc.tile_pool(name="tr", bufs=1))
        axp = ctx.enter_context(tc.tile_pool(name="ax", bufs=2))
        mup = ctx.enter_context(tc.tile_pool(name="mu", bufs=NRC))
        stp = ctx.enter_context(tc.tile_pool(name="st", bufs=1))
        rqp = ctx.enter_context(tc.tile_pool(name="rq", bufs=NRC))
        rqsp = ctx.enter_context(tc.tile_pool(name="rqs", bufs=2))
        zlp = ctx.enter_context(tc.tile_pool(name="zl", bufs=2))
        hp = ctx.enter_context(tc.tile_pool(name="h", bufs=2))
        outp = ctx.enter_context(tc.tile_pool(name="out", bufs=1))

        ztp = ctx.enter_context(tc.tile_pool(name="zt", bufs=4, space="PSUM"))
        spp = ctx.enter_context(tc.tile_pool(name="sp", bufs=1, space="PSUM"))
        bcp = ctx.enter_context(tc.tile_pool(name="bc", bufs=1, space="PSUM"))
        mm2p = ctx.enter_context(tc.tile_pool(name="m2", bufs=2, space="PSUM"))

        # ---- weights / constants into SBUF ----
        w1s = const.tile([P, KC, H], BF16, tag="w1s")
        nc.sync.dma_start(w1s[:], w1.rearrange("(k p) h -> p k h", p=P))
        w2s = const.tile([P, HT, C], BF16, tag="w2s")
        nc.sync.dma_start(w2s[:], w2.rearrange("(c p) n -> p c n", p=P))
        cs2_s = const.tile([2, H], BF16, tag="cs2")
        nc.sync.dma_start(cs2_s[:], cs2[:])
        b1c_s = const.tile([P, HT], F32, tag="b1c")
        nc.sync.dma_start(b1c_s[:], b1c[:])
        b2r_s = const.tile([1, C], BF16, tag="b2r")
        nc.sync.dma_start(b2r_s[:], b2r[:])
        ones_col_s = const.tile([P, 1], BF16, tag="ones_col")
        nc.sync.dma_start(ones_col_s[:], ones_col[:])
        ones_row_b_s = const.tile([1, RCW], BF16, tag="ones_row_b")
        nc.sync.dma_start(ones_row_b_s[:], ones_row_b[:])
        ones_row_f_s = const.tile([1, P], F32, tag="ones_row_f")
        nc.sync.dma_start(ones_row_f_s[:], ones_row_f[:])
        eps_s = const.tile([1, 1], F32, tag="eps")
        nc.vector.memset(eps_s[:], EPS)
        zeros_s = const.tile([P, 1], F32, tag="zeros")
        nc.vector.memset(zeros_s[:], 0.0)
        dum_s = const.tile([1, 1], F32, tag="dum")
        nc.scalar.activation(dum_s[:], eps_s[:], AF.Sqrt, bias=eps_s[:])

        xb = [None] * NRC
        mu_b = [None] * NRC
        sg_b = [None] * NRC
        rsq = [None] * NRC

        # ================= Phase A: loads + LN statistics =================
        for rc in range(NRC):
            xb[rc] = xbp.tile([P, KC, RCW], BF16, tag="xb", name=f"xb{rc}")
            # HBM fp32 -> SBUF bf16 cast during DMA (SWDGE), all issued up front
            src = xT[:, rc * RCW : (rc + 1) * RCW].rearrange("(k p) r -> p k r", p=P)
            nc.gpsimd.dma_start(xb[rc][:], src)

        for rc in range(NRC):
            # squares on ACT ('square' is a filler fn: no table switch)
            x2 = x2p.tile([P, KC, RCW], BF16, tag="x2")
            nc.scalar.activation(x2[:], xb[rc][:], AF.Square)

            # binary-tree partial sums over the 16 k-chunks (DVE, bf16 2x)
            t8 = trp.tile([P, 8, RCW], BF16, tag="t8")
            nc.vector.tensor_add(t8[:], xb[rc][:, 0:8, :], xb[rc][:, 8:16, :])
            t4 = trp.tile([P, 4, RCW], BF16, tag="t4")
            nc.vector.tensor_add(t4[:], t8[:, 0:4, :], t8[:, 4:8, :])
            t2 = trp.tile([P, 2, RCW], BF16, tag="t2")
            nc.vector.tensor_add(t2[:], t4[:, 0:2, :], t4[:, 2:4, :])
            ax = axp.tile([P, RCW], BF16, tag="ax")
            nc.vector.tensor_add(ax[:], t2[:, 0, :], t2[:, 1, :])

            u8 = trp.tile([P, 8, RCW], BF16, tag="u8")
            nc.vector.tensor_add(u8[:], x2[:, 0:8, :], x2[:, 8:16, :])
            u4 = trp.tile([P, 4, RCW], BF16, tag="u4")
            nc.vector.tensor_add(u4[:], u8[:, 0:4, :], u8[:, 4:8, :])
            u2 = trp.tile([P, 2, RCW], BF16, tag="u2")
            nc.vector.tensor_add(u2[:], u4[:, 0:2, :], u4[:, 2:4, :])
            ax2 = axp.tile([P, RCW], BF16, tag="ax2")
            nc.vector.tensor_add(ax2[:], u2[:, 0, :], u2[:, 1, :])

            # partition reduction via ones-matmul: S1 (p0), S2 (p32)
            sp = spp.tile([33, RCW], F32, tag="sp")
            nc.tensor.matmul(sp[0:1, :], ones_col_s[:], ax[:], start=True, stop=True)
            nc.tensor.matmul(sp[32:33, :], ones_col_s[:], ax2[:], start=True, stop=True)

            # finalize: mu (bf16 row), var, sigma=sqrt(var+eps), rsq=1/sigma
            mu_b[rc] = mup.tile([1, RCW], BF16, tag="mu", name=f"mu{rc}")
            nc.vector.tensor_scalar_mul(mu_b[rc][:], sp[0:1, :], 1.0 / D)
            msq = stp.tile([1, RCW], F32, tag="msq")
            nc.vector.tensor_scalar_mul(msq[:], sp[32:33, :], 1.0 / D)
            mu2 = stp.tile([1, RCW], F32, tag="mu2")
            nc.vector.tensor_mul(mu2[:], mu_b[rc][:], mu_b[rc][:])
            var = stp.tile([1, RCW], F32, tag="var")
            nc.vector.tensor_sub(var[:], msq[:], mu2[:])
            sig = stp.tile([1, RCW], F32, tag="sig")
            nc.scalar.activation(sig[:], var[:], AF.Sqrt, bias=eps_s[:])
            if with_b1:
                sg_b[rc] = mup.tile([1, RCW], BF16, tag="sg", name=f"sg{rc}")
                nc.vector.tensor_copy(sg_b[rc][:], sig[:])
            rsq[rc] = rqp.tile([1, RCW], F32, tag="rq", name=f"rq{rc}")
            nc.vector.reciprocal_approx_fast(rsq[rc][:], sig[:])

        # ================= Phase B: MLP =================
        out_t = outp.tile([C, NRC, RCW], F32, tag="out_t")
        for rc in range(NRC):
            # broadcast rsq row to 128 partitions via K=1 matmul (fp32)
            rqB = bcp.tile([P, RCW], F32, tag="rqB")
            nc.tensor.matmul(rqB[:], ones_row_f_s[:], rsq[rc][:], start=True, stop=True)
            rqS = rqsp.tile([P, RCW], F32, tag="rqS")
            nc.scalar.copy(rqS[:], rqB[:])

            zlw = zlp.tile([P, HT, RCW], F32, tag="zlw")
            for ht in range(HT):
                zt = ztp.tile([P, RCW], F32, tag="zt")
                for k in range(KC):
                    nc.tensor.matmul(
                        zt[:],
                        w1s[:, k, ht * P : (ht + 1) * P],
                        xb[rc][:, k, :],
                        start=(k == 0),
                        stop=False,
                    )
                # corrections: += (-colsum) x mu  (+ b1 x sigma)
                nc.tensor.matmul(
                    zt[:],
                    cs2_s[0:1, ht * P : (ht + 1) * P],
                    mu_b[rc][:],
                    start=False,
                    stop=not with_b1,
                )
                if with_b1:
                    nc.tensor.matmul(
                        zt[:],
                        cs2_s[1:2, ht * P : (ht + 1) * P],
                        sg_b[rc][:],
                        start=False,
                        stop=True,
                    )
                nc.vector.tensor_mul(zlw[:, ht, :], zt[:], rqS[:])
            h_w = hp.tile([P, HT, RCW], BF16, tag="h")
            nc.scalar.activation(h_w[:], zlw[:], AF.Gelu, bias=zeros_s[:])

            pp = mm2p.tile([C, RCW], F32, tag="pp")
            for c4 in range(HT):
                nc.tensor.matmul(
                    pp[:], w2s[:, c4, :], h_w[:, c4, :], start=(c4 == 0), stop=False
                )
            nc.tensor.matmul(pp[:], b2r_s[:], ones_row_b_s[:], start=False, stop=True)
            nc.scalar.activation(out_t[:, rc, :], pp[:], AF.Tanh)

        # tanh+1 finalize + single store (add1 in place)
        nc.vector.tensor_scalar_add(out_t[:], out_t[:], 1.0)
        nc.sync.dma_start(pT.rearrange("c (n r) -> c n r", r=RCW), out_t[:])


def _get_nc(with_b1=False):
    key = f"nc{int(with_b1)}"
    if key not in _CACHE:
        _CACHE[key] = _build_nc(with_b1)
    return _CACHE[key]


def _prep_consts(ln_gamma, ln_beta, W1, b1, W2, b2):
    bf16 = ml_dtypes.bfloat16
    W1p = (W1 * ln_gamma[:, None]).astype(np.float32)
    b1p = (b1 + ln_beta @ W1).astype(np.float32)
    return {
        "w1": np.ascontiguousarray(W1p.astype(bf16)),
        "w2": np.ascontiguousarray(W2.astype(bf16)),
        "cs2": np.stack([-W1p.sum(axis=0), b1p]).astype(bf16),
        "b1c": np.ascontiguousarray(b1p.reshape(HT, P).T.astype(np.float32)),
        "b2r": b2.astype(bf16).reshape(1, C),
        "ones_col": np.ones((P, 1), dtype=bf16),
        "ones_row_b": np.ones((1, RCW), dtype=bf16),
        "ones_row_f": np.ones((1, P), dtype=np.float32),
    }


def _run(nc, in_maps, **kw):
    return bass_utils.run_bass_kernel_spmd(
        nc, in_maps, core_ids=list(range(NCORES)), **kw
    )


def kernel(slow_state, ln_gamma, ln_beta, W1, b1, W2, b2, _bench_kw=None):
    slow_state = np.asarray(slow_state, dtype=np.float32)
    b1p_host = np.asarray(b1, np.float32) + np.asarray(ln_beta, np.float32) @ np.asarray(W1, np.float32)
    nc = _get_nc(bool(np.any(b1p_host != 0.0)))
    consts = _prep_consts(
        np.asarray(ln_gamma, np.float32),
        np.asarray(ln_beta, np.float32),
        np.asarray(W1, np.float32),
        np.asarray(b1, np.float32),
        np.asarray(W2, np.float32),
        np.asarray(b2, np.float32),
    )
    in_maps = []
    for c in range(NCORES):
        shard = slow_state[c * BS : (c + 1) * BS, :]
        m = dict(consts)
        m["xT"] = np.ascontiguousarray(shard.T)
        in_maps.append(m)
    res = _run(nc, in_maps, **(_bench_kw or {}))
    if _bench_kw:
        _CACHE["last_result"] = res
    params = np.concatenate(
        [res.results[c]["pT"].T for c in range(NCORES)], axis=0
    )  # [B, C]
    pr = params.reshape(B, NH, 3)
    return (
        np.ascontiguousarray(pr[..., 0]),
        np.ascontiguousarray(pr[..., 1]),
        np.ascontiguousarray(pr[..., 2]),
    )



# revision 3
# speedup vs baseline: 1.0947x; 1.0947x over previous
"""Trainium2 Bass kernel for nn_HOPEProjection (LayerNorm -> MLP(2048->512,
GELU) -> Linear(512->96) -> tanh+1, split into 3 heads-tensors).

Contract: kernel(**inputs) takes the FULL inputs from setup_inputs() and
returns the FULL output (tuple of three [16384, 32] float32 arrays), running
the compute data-parallel across 8 NeuronCores.

v2: software-pipelined over 4 row-chunks of 512 rows per core so the PE
streams matmuls continuously while DVE/ACT compute LayerNorm stats for the
next chunk and the MLP tail for the previous one. Input is host-pre-tiled so
each chunk loads with 128 large contiguous descriptors (SWDGE fp32->bf16
cast). LN stats are replicated across all 128 partitions by a scaled-ones
matmul so the 1/sigma factor needs no extra broadcast matmul.

Self-contained: hardcodes all shapes; does not read any sibling files.
"""

import sys

for _p in ("/opt/trn_rl_repo",):
    if _p not in sys.path:
        sys.path.append(_p)

import numpy as np
import ml_dtypes

import concourse.bacc as bacc
import concourse.mybir as mybir
import concourse.tile as tile
from concourse import bass_utils

# ---- problem constants (hardcoded per contract) ----
P = 128              # SBUF partitions
D = 2048             # d_model
H = 512              # hidden
C = 96               # 3 * n_heads
NH = 32              # n_heads
B = 16384            # batch
NCORES = 8
BS = B // NCORES     # rows per core = 2048
RCW = 512            # row-chunk width (matmul moving dim)
NRC = BS // RCW      # 4 row chunks per core
KC = D // P          # 16 contraction chunks
KG = 4               # k-chunks per DMA group
NKG = KC // KG       # 4 DMA groups per row chunk
HT = H // P          # 4 hidden tiles
EPS = 1e-5

F32 = mybir.dt.float32
BF16 = mybir.dt.bfloat16
AF = mybir.ActivationFunctionType

_CACHE = {}


def _build_nc(with_b1=False):
    nc = bacc.Bacc("TRN2", target_bir_lowering=False, debug=False)

    xt = nc.dram_tensor("xt", [P, NRC * KC, RCW], F32, kind="ExternalInput").ap()
    w1 = nc.dram_tensor("w1", [P, KC * H], BF16, kind="ExternalInput").ap()
    w2 = nc.dram_tensor("w2", [P, HT * C], BF16, kind="ExternalInput").ap()
    cs2 = nc.dram_tensor("cs2", [2, H], BF16, kind="ExternalInput").ap()
    b2r = nc.dram_tensor("b2r", [1, C], BF16, kind="ExternalInput").ap()
    onesr = nc.dram_tensor("onesr", [1, RCW], BF16, kind="ExternalInput").ap()
    onesD = nc.dram_tensor("onesD", [P, P], BF16, kind="ExternalInput").ap()
    pT = nc.dram_tensor("pT", [C, NRC * RCW], F32, kind="ExternalOutput").ap()

    with tile.TileContext(nc) as tc:
        _body(tc, xt, w1, w2, cs2, b2r, onesr, onesD, pT, with_b1)
    nc.compile()
    return nc


def _body(tc, xt, w1, w2, cs2, b2r, onesr, onesD, pT, with_b1):
    nc = tc.nc
    import contextlib

    ctx = contextlib.ExitStack()
    with ctx:
        const = ctx.enter_context(tc.tile_pool(name="const", bufs=1))
        xbp = ctx.enter_context(tc.tile_pool(name="xb", bufs=NRC))
        x2p = ctx.enter_context(tc.tile_pool(name="x2", bufs=2))
        trp = ctx.enter_context(tc.tile_pool(name="tr", bufs=1))
        axp = ctx.enter_context(tc.tile_pool(name="ax", bufs=2))
        mbp = ctx.enter_context(tc.tile_pool(name="mb", bufs=2))
        stp = ctx.enter_context(tc.tile_pool(name="st", bufs=2))
        rqp = ctx.enter_context(tc.tile_pool(name="rq", bufs=2))
        zlp = ctx.enter_context(tc.tile_pool(name="zl", bufs=2))
        hp = ctx.enter_context(tc.tile_pool(name="h", bufs=2))
        outp = ctx.enter_context(tc.tile_pool(name="out", bufs=2))

        ztp = ctx.enter_context(tc.tile_pool(name="zt", bufs=5, space="PSUM"))
        spp = ctx.enter_context(tc.tile_pool(name="sp", bufs=1, space="PSUM"))
        mm2p = ctx.enter_context(tc.tile_pool(name="m2", bufs=1, space="PSUM"))

        # ---- input loads first: SWDGE fp32->bf16 cast, 4 k-groups per rc ----
        xb = [None] * NRC
        for rc in range(NRC):
            xb[rc] = xbp.tile([P, KC, RCW], BF16, tag="xb", name=f"xb{rc}")
            for kg in range(NKG):
                lo = rc * KC + kg * KG
                nc.gpsimd.dma_start(
                    xb[rc][:, kg * KG : (kg + 1) * KG, :],
                    xt[:, lo : lo + KG, :],
                )

        # ---- weights / constants into SBUF (HWDGE, parallel queue) ----
        w1s = const.tile([P, KC, H], BF16, tag="w1s")
        nc.sync.dma_start(w1s[:], w1[:])
        w2s = const.tile([P, HT, C], BF16, tag="w2s")
        nc.sync.dma_start(w2s[:], w2[:])
        cs2_s = const.tile([2, H], BF16, tag="cs2")
        nc.sync.dma_start(cs2_s[:], cs2[:])
        b2r_s = const.tile([1, C], BF16, tag="b2r")
        nc.sync.dma_start(b2r_s[:], b2r[:])
        onesr_s = const.tile([1, RCW], BF16, tag="onesr")
        nc.sync.dma_start(onesr_s[:], onesr[:])
        onesD_s = const.tile([P, P], BF16, tag="onesD")
        nc.sync.dma_start(onesD_s[:], onesD[:])
        eps_s = const.tile([P, 1], F32, tag="eps")
        nc.vector.memset(eps_s[:], EPS)
        zeros_s = const.tile([P, 1], F32, tag="zeros")
        nc.vector.memset(zeros_s[:], 0.0)
        # preload the sqrt activation table during the fill phase
        dum_s = const.tile([1, 1], F32, tag="dum")
        nc.scalar.activation(dum_s[:], eps_s[0:1, :], AF.Sqrt, bias=eps_s[0:1, :])

        mu_b = [None] * NRC
        sg_b = [None] * NRC
        rsq = [None] * NRC
        zt = [[None] * HT for _ in range(NRC)]
        hws = [None] * NRC

        def emit_square_trees(rc):
            # x^2 on ACT ('square' is a filler fn: no table switch)
            x2 = x2p.tile([P, KC, RCW], BF16, tag="x2")
            nc.scalar.activation(x2[:], xb[rc][:], AF.Square)

            # binary-tree partial sums over the 16 k-chunks (DVE, bf16 2x)
            t8 = trp.tile([P, 8, RCW], BF16, tag="t8")
            nc.vector.tensor_add(t8[:], xb[rc][:, 0:8, :], xb[rc][:, 8:16, :])
            t4 = trp.tile([P, 4, RCW], BF16, tag="t4")
            nc.vector.tensor_add(t4[:], t8[:, 0:4, :], t8[:, 4:8, :])
            t2 = trp.tile([P, 2, RCW], BF16, tag="t2")
            nc.vector.tensor_add(t2[:], t4[:, 0:2, :], t4[:, 2:4, :])
            ax = axp.tile([P, RCW], BF16, tag="ax", name=f"ax{rc}")
            nc.vector.tensor_add(ax[:], t2[:, 0, :], t2[:, 1, :])

            u8 = trp.tile([P, 8, RCW], BF16, tag="u8")
            nc.vector.tensor_add(u8[:], x2[:, 0:8, :], x2[:, 8:16, :])
            u4 = trp.tile([P, 4, RCW], BF16, tag="u4")
            nc.vector.tensor_add(u4[:], u8[:, 0:4, :], u8[:, 4:8, :])
            u2 = trp.tile([P, 2, RCW], BF16, tag="u2")
            nc.vector.tensor_add(u2[:], u4[:, 0:2, :], u4[:, 2:4, :])
            ax2 = axp.tile([P, RCW], BF16, tag="ax2", name=f"ax2{rc}")
            nc.vector.tensor_add(ax2[:], u2[:, 0, :], u2[:, 1, :])
            return ax, ax2

        def emit_mm1(rc):
            # main matmuls, k-outer so PSUM banks rotate and weights stream
            for ht in range(HT):
                zt[rc][ht] = ztp.tile([P, RCW], F32, tag="zt", name=f"zt{rc}_{ht}")
            for k in range(KC):
                for ht in range(HT):
                    nc.tensor.matmul(
                        zt[rc][ht][:],
                        w1s[:, k, ht * P : (ht + 1) * P],
                        xb[rc][:, k, :],
                        start=(k == 0),
                        stop=False,
                    )

        def emit_sp(rc, ax, ax2):
            # partition reduction: mu / E[x^2] replicated on all 128 partitions
            muP = spp.tile([P, RCW], F32, tag="muP", name=f"muP{rc}")
            nc.tensor.matmul(muP[:], onesD_s[:], ax[:], start=True, stop=True)
            msqP = spp.tile([P, RCW], F32, tag="msqP", name=f"msqP{rc}")
            nc.tensor.matmul(msqP[:], onesD_s[:], ax2[:], start=True, stop=True)
            return muP, msqP

        def emit_finalize(rc, muP, msqP):
            # mu row (bf16, moving operand of the correction matmul)
            mu_b[rc] = mbp.tile([1, RCW], BF16, tag="mu", name=f"mu{rc}")
            nc.scalar.copy(mu_b[rc][:], muP[0:1, :])
            # var = E[x^2] - mu^2 ; sig = sqrt(var+eps) ; rsq = 1/sig
            mu2 = stp.tile([P, RCW], F32, tag="mu2")
            nc.scalar.activation(mu2[:], muP[:], AF.Square)
            var = stp.tile([P, RCW], F32, tag="var")
            nc.vector.tensor_sub(var[:], msqP[:], mu2[:])
            sig = stp.tile([P, RCW], F32, tag="sig")
            nc.scalar.activation(sig[:], var[:], AF.Sqrt, bias=eps_s[:])
            if with_b1:
                sg_b[rc] = mbp.tile([1, RCW], BF16, tag="sg", name=f"sg{rc}")
                nc.scalar.copy(sg_b[rc][:], sig[0:1, :])
            rsq[rc] = rqp.tile([P, RCW], F32, tag="rq", name=f"rq{rc}")
            nc.vector.reciprocal_approx_fast(rsq[rc][:], sig[:])

        def emit_corr(rc):
            for ht in range(HT):
                nc.tensor.matmul(
                    zt[rc][ht][:],
                    cs2_s[0:1, ht * P : (ht + 1) * P],
                    mu_b[rc][:],
                    start=False,
                    stop=not with_b1,
                )
                if with_b1:
                    nc.tensor.matmul(
                        zt[rc][ht][:],
                        cs2_s[1:2, ht * P : (ht + 1) * P],
                        sg_b[rc][:],
                        start=False,
                        stop=True,
                    )

        def emit_norm_gelu(rc):
            zl = zlp.tile([P, HT, RCW], BF16, tag="zl")
            for ht in range(HT):
                nc.vector.tensor_mul(zl[:, ht, :], zt[rc][ht][:], rsq[rc][:])
            hws[rc] = hp.tile([P, HT, RCW], BF16, tag="h", name=f"h{rc}")
            nc.scalar.activation(hws[rc][:], zl[:], AF.Gelu, bias=zeros_s[:])

        def emit_out(rc):
            pp = mm2p.tile([C, RCW], F32, tag="pp")
            for c4 in range(HT):
                nc.tensor.matmul(
                    pp[:], w2s[:, c4, :], hws[rc][:, c4, :],
                    start=(c4 == 0), stop=False,
                )
            nc.tensor.matmul(pp[:], b2r_s[:], onesr_s[:], start=False, stop=True)
            ot = outp.tile([C, RCW], F32, tag="ot")
            nc.scalar.activation(ot[:], pp[:], AF.Tanh)
            nc.vector.tensor_scalar_add(ot[:], ot[:], 1.0)
            nc.sync.dma_start(pT[:, rc * RCW : (rc + 1) * RCW], ot[:])

        for rc in range(NRC):
            ax, ax2 = emit_square_trees(rc)
            emit_mm1(rc)
            muP, msqP = emit_sp(rc, ax, ax2)
            if rc > 0:
                emit_out(rc - 1)
            emit_finalize(rc, muP, msqP)
            emit_corr(rc)
            emit_norm_gelu(rc)
        emit_out(NRC - 1)


def _get_nc(with_b1=False):
    key = f"nc{int(with_b1)}"
    if key not in _CACHE:
        _CACHE[key] = _build_nc(with_b1)
    return _CACHE[key]


def _prep_consts(ln_gamma, ln_beta, W1, b1, W2, b2):
    bf16 = ml_dtypes.bfloat16
    W1p = (W1 * ln_gamma[:, None]).astype(np.float32)
    b1p = (b1 + ln_beta @ W1).astype(np.float32)
    w1t = np.ascontiguousarray(
        W1p.reshape(KC, P, H).transpose(1, 0, 2)
    ).reshape(P, KC * H)
    w2t = np.ascontiguousarray(
        W2.reshape(HT, P, C).transpose(1, 0, 2)
    ).reshape(P, HT * C)
    return {
        "w1": w1t.astype(bf16),
        "w2": w2t.astype(bf16),
        "cs2": np.stack([-W1p.sum(axis=0), b1p]).astype(bf16),
        "b2r": b2.astype(bf16).reshape(1, C),
        "onesr": np.ones((1, RCW), dtype=bf16),
        "onesD": np.full((P, P), 1.0 / D, dtype=bf16),
    }


def _run(nc, in_maps, **kw):
    return bass_utils.run_bass_kernel_spmd(
        nc, in_maps, core_ids=list(range(NCORES)), **kw
    )


def kernel(slow_state, ln_gamma, ln_beta, W1, b1, W2, b2, _bench_kw=None):
    slow_state = np.asarray(slow_state, dtype=np.float32)
    b1p_host = np.asarray(b1, np.float32) + np.asarray(ln_beta, np.float32) @ np.asarray(W1, np.float32)
    nc = _get_nc(bool(np.any(b1p_host != 0.0)))
    consts = _prep_consts(
        np.asarray(ln_gamma, np.float32),
        np.asarray(ln_beta, np.float32),
        np.asarray(W1, np.float32),
        np.asarray(b1, np.float32),
        np.asarray(W2, np.float32),
        np.asarray(b2, np.float32),
    )
    in_maps = []
    for c in range(NCORES):
        shard = slow_state[c * BS : (c + 1) * BS, :]
        # [p, rc, k, r] = shard[rc*RCW + r, k*P + p], contiguous per (p, rc)
        xprep = np.ascontiguousarray(
            shard.reshape(NRC, RCW, KC, P).transpose(3, 0, 2, 1)
        ).reshape(P, NRC * KC, RCW)
        m = dict(consts)
        m["xt"] = xprep
        in_maps.append(m)
    res = _run(nc, in_maps, **(_bench_kw or {}))
    if _bench_kw:
        _CACHE["last_result"] = res
    params = np.concatenate(
        [res.results[c]["pT"].T for c in range(NCORES)], axis=0
    )  # [B, C]
    pr = params.reshape(B, NH, 3)
    return (
        np.ascontiguousarray(pr[..., 0]),
        np.ascontiguousarray(pr[..., 1]),
        np.ascontiguousarray(pr[..., 2]),
    )
